# revision 3
# baseline (speedup 1.0000x reference)
"""Trainium2 Bass kernel for nn_Attention_43413529428606 (linear attention
with l2-normed q/k, interleaved RoPE, mask, per-head power scaling).

Sharding: the 16384 (batch*seq) rows are split across 8 NeuronCores, 2048
rows each; cores 0-3 take batch 0, cores 4-7 batch 1.  Each core computes
q/k/v projections for its rows (all 16 heads), applies l2norm+RoPE+mask,
accumulates the per-head k^T v state, AllReduces that state (512 KB) within
its batch group, then applies attention and the output projection for its
rows.  The data path is fp16 (fp32 PSUM accumulation); q/attn/out phases
are fused per 512-row supertile so nothing spills to DRAM.  The q-side
mask is applied host-side on the output rows.

Self-contained: hardcodes all shapes; no sibling imports.
"""

import sys

for _p in ("/opt/trn_rl_repo",):
    if _p not in sys.path:
        sys.path.append(_p)

from contextlib import ExitStack

import numpy as np

import concourse.bass as bass
import concourse.bacc as bacc
import concourse.tile as tile
from concourse import mybir
from concourse.bass_utils import run_bass_kernel_spmd

F32 = mybir.dt.float32
F16 = mybir.dt.float16

DIM = 1024
H = 16
HD = 64
B = 2
C = 8192
ROPE_THETA = 10000.0

N_CORES = 8
R = (B * C) // N_CORES  # 2048 rows per core
NC_T = R // 128  # 16 c-tiles of 128 (phase A)
NQ_T = R // 512  # 4 c-supertiles of 512 (fused q/attn/out phase)
ND = DIM // 128  # 8 d-chunks
NJ = DIM // 128  # 8 j-tiles
NPAIR = H // 2  # 8 head pairs

Copy = mybir.ActivationFunctionType.Copy
Square = mybir.ActivationFunctionType.Square
Ln = mybir.ActivationFunctionType.Ln
Exp = mybir.ActivationFunctionType.Exp
MUL = mybir.AluOpType.mult
ADD = mybir.AluOpType.add


def build_nc(sim_mode=False, phases="ABC", reps=1, no_ar=False):
    nc = bacc.Bacc(
        "TRN2",
        target_bir_lowering=False,
        debug=False,
        num_devices=1 if sim_mode else N_CORES,
    )

    # ---- DRAM parameters (per-core shapes, fp16 data path) ----
    xT = nc.dram_tensor("xT", [DIM, R], F16, kind="ExternalInput").ap()
    WkT = nc.dram_tensor("WkT", [DIM, DIM], F16, kind="ExternalInput").ap()
    WvT = nc.dram_tensor("WvT", [DIM, DIM], F16, kind="ExternalInput").ap()
    WqT = nc.dram_tensor("WqT", [DIM, DIM], F16, kind="ExternalInput").ap()
    WoT = nc.dram_tensor("WoT", [DIM, DIM], F16, kind="ExternalInput").ap()
    cosC = nc.dram_tensor("cosC", [R, HD], F16, kind="ExternalInput").ap()
    sinC = nc.dram_tensor("sinC", [R, HD], F16, kind="ExternalInput").ap()
    cosF = nc.dram_tensor("cosF", [128, R], F16, kind="ExternalInput").ap()
    sinF = nc.dram_tensor("sinF", [128, R], F16, kind="ExternalInput").ap()
    maskC = nc.dram_tensor("maskC", [128, NC_T], F32, kind="ExternalInput").ap()
    ind16T = nc.dram_tensor("ind16T", [DIM, 16], F16, kind="ExternalInput").ap()
    ind16 = nc.dram_tensor("ind16", [16, DIM], F16, kind="ExternalInput").ap()
    Pmat = nc.dram_tensor("Pmat", [128, 128], F16, kind="ExternalInput").ap()

    kv_in_d = nc.dram_tensor("kv_in_d", [128, NPAIR * 128], F32)
    kv_out_d = nc.dram_tensor("kv_out_d", [128, NPAIR * 128], F32)

    out_d = nc.dram_tensor("out", [DIM, R], F32, kind="ExternalOutput").ap()

    def blkview(dram_ap, csl):
        return dram_ap.rearrange("(t p) c -> p t c", p=128)[:, :, csl]

    with tile.TileContext(nc) as tc:
        with ExitStack() as ctx:
            consts = ctx.enter_context(tc.tile_pool(name="consts", bufs=1))
            kvblk_pool = ctx.enter_context(tc.tile_pool(name="kvblk", bufs=1))

            cosC_t = consts.tile([128, NC_T * HD], F16, tag="cosC")
            sinC_t = consts.tile([128, NC_T * HD], F16, tag="sinC")
            nc.sync.dma_start(
                out=cosC_t[:].rearrange("p (t f) -> p t f", t=NC_T),
                in_=cosC[:].rearrange("(t p) f -> p t f", p=128),
            )
            nc.sync.dma_start(
                out=sinC_t[:].rearrange("p (t f) -> p t f", t=NC_T),
                in_=sinC[:].rearrange("(t p) f -> p t f", p=128),
            )
            maskC_t = consts.tile([128, NC_T], F32, tag="maskC")
            ind16T_t = consts.tile([128, NJ * 16], F16, tag="ind16T")
            ind16_t = consts.tile([16, DIM], F16, tag="ind16")
            P_t = consts.tile([128, 128], F16, tag="Pmat")
            nc.sync.dma_start(out=maskC_t[:], in_=maskC[:])
            nc.sync.dma_start(
                out=ind16T_t[:].rearrange("p (t f) -> p t f", t=NJ),
                in_=ind16T[:].rearrange("(t p) f -> p t f", p=128),
            )
            nc.sync.dma_start(out=ind16_t[:], in_=ind16[:])
            nc.sync.dma_start(out=P_t[:], in_=Pmat[:])

            for _rep in range(reps):
              with ExitStack() as ctxX:
                xpool = ctxX.enter_context(tc.tile_pool(name="xpool", bufs=1))
                xT_all = xpool.tile([128, ND * R], F16, tag="xT")
                for xc in range(4):
                    nc.sync.dma_start(
                        out=xT_all[:, xc * 2 * R : (xc + 1) * 2 * R].rearrange(
                            "p (t c) -> p t c", t=2
                        ),
                        in_=xT[xc * 256 : (xc + 1) * 256, :].rearrange(
                            "(t p) c -> p t c", p=128
                        ),
                    )

                def xsl(dc, csl):
                    lo = dc * R
                    return xT_all[:, lo + csl.start : lo + csl.stop]

                if "B" in phases and "C" in phases:
                    wBC = ctxX.enter_context(tc.tile_pool(name="wBC", bufs=1))
                    wq_all = wBC.tile([128, ND * DIM], F16, tag="wq")
                    nc.sync.dma_start(
                        out=wq_all[:].rearrange("p (t f) -> p t f", t=ND),
                        in_=WqT[:].rearrange("(t p) f -> p t f", p=128),
                    )
                    wo_all = wBC.tile([128, ND * DIM], F16, tag="wo")
                    nc.scalar.dma_start(
                        out=wo_all[:].rearrange("p (t f) -> p t f", t=ND),
                        in_=WoT[:].rearrange("(t p) f -> p t f", p=128),
                    )
                    cosF_t = wBC.tile([128, R], F16, tag="cosF")
                    sinF_t = wBC.tile([128, R], F16, tag="sinF")
                    nc.sync.dma_start(out=cosF_t[:], in_=cosF[:])
                    nc.scalar.dma_start(out=sinF_t[:], in_=sinF[:])

                # ========= Phase A: k/v proj + process + kv Grams ==========
                with ExitStack() as ctxA:
                  if "A" in phases:
                    wA = ctxA.enter_context(tc.tile_pool(name="wA", bufs=1))
                    psA = ctxA.enter_context(
                        tc.tile_pool(name="psA", bufs=3, space="PSUM")
                    )
                    pskv = ctxA.enter_context(
                        tc.tile_pool(name="pskv", bufs=1, space="PSUM")
                    )
                    sbA = ctxA.enter_context(tc.tile_pool(name="sbA", bufs=2))
                    sb1 = ctxA.enter_context(tc.tile_pool(name="sb1", bufs=2))
                    smA = ctxA.enter_context(tc.tile_pool(name="smA", bufs=2))

                    wk_all = wA.tile([128, ND * DIM], F16, tag="wk")
                    wv_all = wA.tile([128, ND * DIM], F16, tag="wv")
                    for wt, wsrc in ((wk_all, WkT), (wv_all, WvT)):
                        for xc in range(2):
                            nc.scalar.dma_start(
                                out=wt[
                                    :, xc * 4 * DIM : (xc + 1) * 4 * DIM
                                ].rearrange("p (t f) -> p t f", t=4),
                                in_=wsrc[xc * 512 : (xc + 1) * 512, :].rearrange(
                                    "(t p) f -> p t f", p=128
                                ),
                            )

                    kv_ps = pskv.tile([128, NPAIR * 128], F32, tag="kvps")
                    kv_pending = []

                    # On HW start=True zeroes the whole PSUM bank, so only
                    # the first pair written to each bank may carry it.
                    def _emit_kv(item):
                        ct_, khat_, v_ = item
                        for p in range(NPAIR):
                            ps_ = slice(p * 128, (p + 1) * 128)
                            nc.tensor.matmul(
                                kv_ps[:, ps_],
                                khat_[:, ps_],
                                v_[:, ps_],
                                start=(
                                    True
                                    if sim_mode
                                    else (ct_ == 0 and p % 4 == 0)
                                ),
                                stop=(
                                    True if sim_mode else (ct_ == NC_T - 1)
                                ),
                            )

                    for ct in range(NC_T):
                        cs = slice(ct * 128, (ct + 1) * 128)
                        k_ps = psA.tile([128, DIM], F32, tag="proj_ps")
                        v_ps = psA.tile([128, DIM], F32, tag="proj_ps")
                        for half in range(2):
                            js = slice(half * 512, (half + 1) * 512)
                            for dc in range(ND):
                                nc.tensor.matmul(
                                    k_ps[:, js],
                                    xsl(dc, cs),
                                    wk_all[
                                        :, dc * DIM + js.start : dc * DIM + js.stop
                                    ],
                                    start=(dc == 0),
                                    stop=(dc == ND - 1),
                                )
                            for dc in range(ND):
                                nc.tensor.matmul(
                                    v_ps[:, js],
                                    xsl(dc, cs),
                                    wv_all[
                                        :, dc * DIM + js.start : dc * DIM + js.stop
                                    ],
                                    start=(dc == 0),
                                    stop=(dc == ND - 1),
                                )

                        # v: evict with mask fold (per-partition scale), cast
                        v_sb = sbA.tile([128, DIM], F16, tag="v_sb")
                        nc.scalar.activation(
                            v_sb[:], v_ps[:], Copy, scale=maskC_t[:, ct : ct + 1]
                        )
                        # k: evict fast (cast fp16) to free the PSUM slot
                        k_sb = sbA.tile([128, DIM], F16, tag="k_sb")
                        nc.scalar.activation(k_sb[:], k_ps[:], Copy)
                        # squares for the l2 norm
                        sq = sbA.tile([128, DIM], F16, tag="sq")
                        nc.scalar.activation(sq[:], k_ps[:], Square)

                        cosb = (
                            cosC_t[:, ct * HD : (ct + 1) * HD]
                            .unsqueeze(1)
                            .broadcast_to([128, H, HD])
                        )
                        sinb4 = (
                            sinC_t[:, ct * HD : (ct + 1) * HD]
                            .rearrange("p (g two) -> p g two", two=2)
                            .unsqueeze(1)
                            .broadcast_to([128, H, HD // 2, 2])
                        )
                        k3 = k_sb[:].rearrange("p (h f) -> p h f", h=H)
                        k_sw = k_sb[:].rearrange(
                            "p (h g two) -> p h g two", h=H, two=2
                        )[:, :, :, ::-1]

                        m1 = sb1.tile([128, DIM], F16, tag="m1")
                        nc.vector.tensor_tensor(
                            m1[:].rearrange("p (h f) -> p h f", h=H), k3, cosb, MUL
                        )
                        red = smA.tile([128, H], F32, tag="red")
                        nc.vector.tensor_reduce(
                            red[:],
                            sq[:].rearrange("p (h f) -> p h f", h=H),
                            mybir.AxisListType.X,
                            ADD,
                        )
                        lnr = smA.tile([128, H], F32, tag="lnr")
                        nc.scalar.activation(lnr[:], red[:], Ln)
                        rs = smA.tile([128, H], F32, tag="rs")
                        nc.scalar.activation(rs[:], lnr[:], Exp, scale=-0.5)
                        rsm = smA.tile([128, H], F32, tag="rsm")
                        nc.vector.tensor_scalar_mul(
                            rsm[:], rs[:], maskC_t[:, ct : ct + 1]
                        )
                        m2 = sb1.tile([128, DIM], F16, tag="m2")
                        nc.vector.tensor_tensor(
                            m2[:].rearrange("p (h g two) -> p h g two", h=H, two=2),
                            k_sw,
                            sinb4,
                            MUL,
                        )
                        s = sb1.tile([128, DIM], F16, tag="s")
                        nc.vector.tensor_tensor(s[:], m1[:], m2[:], ADD)
                        khat = sbA.tile([128, DIM], F16, tag="khat")
                        rsb = rsm[:].unsqueeze(2).broadcast_to([128, H, HD])
                        nc.vector.tensor_tensor(
                            khat[:].rearrange("p (h f) -> p h f", h=H),
                            s[:].rearrange("p (h f) -> p h f", h=H),
                            rsb,
                            MUL,
                        )

                        # kv Grams are issued one iteration late (software
                        # pipelining) so PE never waits on the khat chain
                        kv_pending.append((ct, khat, v_sb))
                        if len(kv_pending) > 1:
                            _emit_kv(kv_pending.pop(0))

                    while kv_pending:
                        _emit_kv(kv_pending.pop(0))

                    # evict kv partials and run the collective
                    kv_sb = sbA.tile([128, NPAIR * 128], F32, tag="kv_sb")
                    nc.vector.tensor_copy(kv_sb[:], kv_ps[:])
                    nc.sync.dma_start(out=kv_in_d.ap(), in_=kv_sb[:])
                    if sim_mode or no_ar:
                        # stand-in for the AllReduce so TimelineSim can run
                        nc.sync.dma_start(out=kv_out_d.ap(), in_=kv_in_d.ap())
                    else:
                        nc.gpsimd.collective_compute(
                            "AllReduce",
                            ADD,
                            replica_groups=[[0, 1, 2, 3], [4, 5, 6, 7]],
                            ins=[kv_in_d.ap().opt()],
                            outs=[kv_out_d.ap().opt()],
                        )

                # kvblk: load reduced Grams, cast to fp16 block-diag
                kvblk = kvblk_pool.tile([128, NPAIR * 128], F16, tag="kvblk")
                if "C" in phases:
                    kvf = kvblk_pool.tile([128, NPAIR * 128], F32, tag="kvf")
                    nc.scalar.dma_start(out=kvf[:], in_=kv_out_d.ap())
                    nc.vector.memset(kvblk[:], 0.0)
                    # top-left diag blocks of each pair, then bottom-right
                    nc.vector.tensor_copy(
                        kvblk[0:64, :].rearrange("p (t f) -> p t f", t=NPAIR)[
                            :, :, 0:64
                        ],
                        kvf[0:64, :].rearrange("p (t f) -> p t f", t=NPAIR)[
                            :, :, 0:64
                        ],
                    )
                    nc.vector.tensor_copy(
                        kvblk[64:128, :].rearrange("p (t f) -> p t f", t=NPAIR)[
                            :, :, 64:128
                        ],
                        kvf[64:128, :].rearrange("p (t f) -> p t f", t=NPAIR)[
                            :, :, 64:128
                        ],
                    )

                # ==== Fused phase B+C: q proj/norm/rope + attn + out proj ===
                with ExitStack() as ctxB:
                  if "B" in phases and "C" in phases:
                    psB = ctxB.enter_context(
                        tc.tile_pool(name="psB", bufs=2, space="PSUM")
                    )
                    psN = ctxB.enter_context(
                        tc.tile_pool(name="psN", bufs=1, space="PSUM")
                    )
                    psAt = ctxB.enter_context(
                        tc.tile_pool(name="psAt", bufs=1, space="PSUM")
                    )
                    psO = ctxB.enter_context(
                        tc.tile_pool(name="psO", bufs=2, space="PSUM")
                    )
                    sbB = ctxB.enter_context(tc.tile_pool(name="sbB", bufs=3))
                    sbS = ctxB.enter_context(
                        tc.tile_pool(name="sbS", bufs=2 * NJ)
                    )
                    sbQ = ctxB.enter_context(tc.tile_pool(name="sbQ", bufs=2))
                    sbAt = ctxB.enter_context(
                        tc.tile_pool(name="sbAt", bufs=NJ + 2)
                    )

                    def _emit_attn_out(item):
                        ct_, qh_ = item
                        cs_ = slice(ct_ * 512, (ct_ + 1) * 512)
                        attn_sb = []
                        for hp in range(NPAIR):
                            a_ps = psAt.tile([128, 512], F32, tag="a_ps")
                            nc.tensor.matmul(
                                a_ps[:],
                                kvblk[:, hp * 128 : (hp + 1) * 128],
                                qh_[:, hp * 512 : (hp + 1) * 512],
                                start=True,
                                stop=True,
                            )
                            a_sb = sbAt.tile([128, 512], F16, tag="a_sb")
                            if hp % 2 == 0:
                                nc.scalar.activation(a_sb[:], a_ps[:], Copy)
                            else:
                                nc.vector.tensor_copy(a_sb[:], a_ps[:])
                            attn_sb.append(a_sb)

                        o_all = sbQ.tile([128, NJ * 512], F32, tag="o_all")
                        for et in range(NJ):
                            elo = et * 128
                            o_ps = psO.tile([128, 512], F32, tag="o_ps")
                            for jt in range(NJ):
                                nc.tensor.matmul(
                                    o_ps[:],
                                    wo_all[
                                        :, jt * DIM + elo : jt * DIM + elo + 128
                                    ],
                                    attn_sb[jt][:],
                                    start=(jt == 0),
                                    stop=(jt == NJ - 1),
                                )
                            nc.scalar.activation(
                                o_all[:, et * 512 : (et + 1) * 512], o_ps[:], Copy
                            )
                        nc.scalar.dma_start(
                            out=blkview(out_d, cs_),
                            in_=o_all[:].rearrange("p (t c) -> p t c", t=NJ),
                        )

                    at_pending = []
                    for ct in range(NQ_T):
                        cs = slice(ct * 512, (ct + 1) * 512)
                        norms_ps = psN.tile([16, 512], F32, tag="norms")
                        qh_all = sbQ.tile([128, NJ * 512], F16, tag="qhall")
                        q_sbs = []
                        # pass 1: projections + squares + norm accumulation
                        for jt in range(NJ):
                            jlo = jt * 128
                            q_ps = psB.tile([128, 512], F32, tag="q_ps")
                            for dc in range(ND):
                                nc.tensor.matmul(
                                    q_ps[:],
                                    wq_all[
                                        :, dc * DIM + jlo : dc * DIM + jlo + 128
                                    ],
                                    xsl(dc, cs),
                                    start=(dc == 0),
                                    stop=(dc == ND - 1),
                                )
                            q_sb = sbS.tile([128, 512], F16, tag="q_sb")
                            nc.scalar.activation(q_sb[:], q_ps[:], Copy)
                            sq = sbB.tile([128, 512], F16, tag="sqB")
                            nc.vector.tensor_mul(sq[:], q_sb[:], q_sb[:])
                            nc.tensor.matmul(
                                norms_ps[:],
                                ind16T_t[:, jt * 16 : (jt + 1) * 16],
                                sq[:],
                                start=(jt == 0),
                                stop=(jt == NJ - 1),
                            )
                            q_sbs.append(q_sb)

                        lnn = sbB.tile([16, 512], F32, tag="lnn")
                        nc.scalar.activation(lnn[:], norms_ps[:], Ln)
                        rs16 = sbB.tile([16, 512], F16, tag="rs16")
                        nc.scalar.activation(rs16[:], lnn[:], Exp, scale=-0.5)

                        # pass 2: rotation + rope + scale into qh_all
                        for jt in range(NJ):
                            q_sb = q_sbs[jt]
                            rot_ps = psB.tile([128, 512], F32, tag="rotrep")
                            nc.tensor.matmul(
                                rot_ps[:], P_t[:], q_sb[:], start=True, stop=True
                            )
                            rep_ps = psB.tile([128, 512], F32, tag="rotrep")
                            nc.tensor.matmul(
                                rep_ps[:],
                                ind16_t[:, jt * 128 : (jt + 1) * 128],
                                rs16[:],
                                start=True,
                                stop=True,
                            )
                            t1 = sbB.tile([128, 512], F16, tag="t1")
                            nc.vector.tensor_tensor(
                                t1[:], q_sb[:], cosF_t[:, cs], MUL
                            )
                            t2 = sbB.tile([128, 512], F16, tag="t2")
                            nc.vector.tensor_tensor(
                                t2[:], rot_ps[:], sinF_t[:, cs], MUL
                            )
                            s = sbB.tile([128, 512], F16, tag="sB")
                            nc.vector.tensor_tensor(s[:], t1[:], t2[:], ADD)
                            nc.vector.tensor_tensor(
                                qh_all[:, jt * 512 : (jt + 1) * 512],
                                s[:],
                                rep_ps[:],
                                MUL,
                            )

                        at_pending.append((ct, qh_all))
                        if len(at_pending) > 1:
                            _emit_attn_out(at_pending.pop(0))

                    while at_pending:
                        _emit_attn_out(at_pending.pop(0))

    nc.compile()
    return nc


_NC_CACHE = None


def _get_nc():
    global _NC_CACHE
    if _NC_CACHE is None:
        _NC_CACHE = build_nc()
    return _NC_CACHE


def make_in_maps(x, mask, Wq, Wk, Wv, Wo, norm_const):
    x = np.asarray(x, np.float32)
    mask = np.asarray(mask)
    Wq = np.asarray(Wq, np.float32)
    Wk = np.asarray(Wk, np.float32)
    Wv = np.asarray(Wv, np.float32)
    Wo = np.asarray(Wo, np.float32)
    norm_const = np.asarray(norm_const, np.float32).reshape(H)

    sig = 1.0 / (1.0 + np.exp(-norm_const.astype(np.float64)))
    svec = np.float64(C) ** (-sig)  # [H]
    s_cols = np.repeat(svec, HD)  # [DIM]

    f16 = np.float16
    WkT = np.ascontiguousarray(Wk.T).astype(f16)
    WvT = np.ascontiguousarray((Wv * s_cols[:, None].astype(np.float32)).T).astype(
        f16
    )
    WqT = np.ascontiguousarray(Wq.T).astype(f16)
    WoT = np.ascontiguousarray(Wo.T).astype(f16)

    inv_freq = 1.0 / (
        ROPE_THETA ** (np.arange(0, HD, 2, dtype=np.float64) / HD)
    )  # [32]
    freq_of_j = np.repeat(inv_freq, 2)  # [64] interleaved

    ind16T = np.zeros((DIM, 16), f16)
    for jt in range(NJ):
        for kk in range(128):
            ind16T[jt * 128 + kk, 2 * jt + (kk >= 64)] = 1.0

    ind16 = np.zeros((16, DIM), f16)
    for jt in range(NJ):
        for m in range(128):
            ind16[2 * jt + (m >= 64), jt * 128 + m] = 1.0

    Pmat = np.zeros((128, 128), f16)
    for i in range(64):
        Pmat[2 * i + 1, 2 * i] = -1.0  # out[2i] = -q[2i+1]
        Pmat[2 * i, 2 * i + 1] = 1.0  # out[2i+1] = q[2i]

    in_maps = []
    for core in range(N_CORES):
        b = core // (N_CORES // B)
        cc = core % (N_CORES // B)
        c0 = cc * R
        pos = (c0 + np.arange(R)).astype(np.float64)

        xTc = np.ascontiguousarray(x[b, c0 : c0 + R, :].T).astype(f16)

        angC = pos[:, None] * freq_of_j[None, :]  # [R, 64]
        cosCc = np.cos(angC).astype(f16)
        sinCc = np.sin(angC).astype(np.float32)
        # sign fold for the swap formulation: even j -> -sin, odd j -> +sin
        sinCc[:, 0::2] *= -1.0
        sinCc = sinCc.astype(f16)

        angF = freq_of_j[:, None] * pos[None, :]  # [64, R]
        angF2 = np.concatenate([angF, angF], axis=0)  # [128, R]
        cosFc = np.cos(angF2).astype(f16)
        sinFc = np.sin(angF2).astype(f16)

        mrow = mask[b, c0 : c0 + R].astype(np.float32)  # [R]
        maskCc = np.ascontiguousarray(mrow.reshape(NC_T, 128).T)  # [128, NC_T]

        in_maps.append(
            {
                "xT": xTc,
                "WkT": WkT,
                "WvT": WvT,
                "WqT": WqT,
                "WoT": WoT,
                "cosC": cosCc,
                "sinC": sinCc,
                "cosF": cosFc,
                "sinF": sinFc,
                "maskC": maskCc,
                "ind16T": ind16T,
                "ind16": ind16,
                "Pmat": Pmat,
            }
        )
    return in_maps


def assemble_output(results, mask):
    out = np.empty((B, C, DIM), np.float32)
    for core in range(N_CORES):
        b = core // (N_CORES // B)
        cc = core % (N_CORES // B)
        c0 = cc * R
        out[b, c0 : c0 + R, :] = results[core]["out"].T
    # q-side mask: masked rows produce zero output
    out *= np.asarray(mask)[:, :, None].astype(np.float32)
    return out


def kernel(x, mask, Wq, Wk, Wv, Wo, norm_const):
    nc = _get_nc()
    in_maps = make_in_maps(x, mask, Wq, Wk, Wv, Wo, norm_const)
    res = run_bass_kernel_spmd(nc, in_maps, list(range(N_CORES)))
    return assemble_output(res.results, mask)



# revision 54
# speedup vs baseline: 3.5385x; 3.5385x over previous
"""Trainium2 Bass kernel for nn_Attention_43413529428606 (linear attention
with l2-normed q/k, interleaved RoPE, mask, per-head power scaling).

Sharding: the 16384 (batch*seq) rows are split across 8 NeuronCores, 2048
rows each; cores 0-3 take batch 0, cores 4-7 batch 1.  Each core computes
k/v projections for its rows (all 16 heads), applies l2norm+RoPE+mask,
accumulates the per-head v^T k Gram, AllReduces the packed diagonal blocks
(256 KB) within its batch group, folds Wo into the reduced Gram
(G = blockdiag(vk) @ Wo^T), and finishes with out = G^T q — the attention
and output projection collapse into one GEMM.  The q-side phase (proj +
l2norm + RoPE) is emitted before anything that depends on the AllReduce and
all AllReduce-dependent loads/copies ride the gpsimd queue, so the
collective overlaps the whole q phase instead of stalling the engines.
The data path is fp16 (fp32 PSUM accumulation); only {Copy, Square, Rsqrt}
activation functions are used so a single act table load suffices.  The
q-side mask is applied host-side on the output rows.

Self-contained: hardcodes all shapes; no sibling imports.
"""

import sys

for _p in ("/opt/trn_rl_repo",):
    if _p not in sys.path:
        sys.path.append(_p)

from contextlib import ExitStack

import numpy as np

import concourse.bass as bass
import concourse.bacc as bacc
import concourse.tile as tile
from concourse import mybir
from concourse.bass_utils import run_bass_kernel_spmd

F32 = mybir.dt.float32
F16 = mybir.dt.float16
F8 = mybir.dt.float8e4
DoubleRow = mybir.MatmulPerfMode.DoubleRow

# power-of-two pre-scales for the fp8 data path (exactly invertible)
XS_K = 32.0  # Wk/Wq host upscale; cancels in the l2 norm
XS_V = 8192.0  # Wv host upscale; descaled in the v eviction
XS_G = 64.0  # G eviction upscale into fp8
XS_Q = 8.0  # qhat upscale into fp8 (folded into rs16)

DIM = 1024
H = 16
HD = 64
B = 2
C = 8192
ROPE_THETA = 10000.0

N_CORES = 8
R = (B * C) // N_CORES  # 2048 rows per core
NC_T = R // 128  # 16 c-tiles of 128 (phase A)
NQ_T = R // 512  # 4 c-supertiles of 512 (q and out phases)
ND = DIM // 128  # 8 d-chunks
NJ = DIM // 128  # 8 j-tiles
NPAIR = H // 2  # 8 head pairs

Copy = mybir.ActivationFunctionType.Copy
Square = mybir.ActivationFunctionType.Square
Sqrt = mybir.ActivationFunctionType.Sqrt
MUL = mybir.AluOpType.mult
ADD = mybir.AluOpType.add


def build_nc(
    sim_mode=False, phases="ABC", reps=1, no_ar=False, fp8=False, fp8o=False
):
    nc = bacc.Bacc(
        "TRN2",
        target_bir_lowering=False,
        debug=False,
        num_devices=1 if sim_mode else N_CORES,
    )

    # ---- DRAM parameters (per-core shapes, fp8/fp16 data path) ----
    FP = F8 if fp8 else F16
    xT = nc.dram_tensor("xT8" if fp8 else "xT", [DIM, R], FP, kind="ExternalInput").ap()
    WkT = nc.dram_tensor("Wk8" if fp8 else "WkT", [DIM, DIM], FP, kind="ExternalInput").ap()
    WvT = nc.dram_tensor("Wv8" if fp8 else "WvT", [DIM, DIM], FP, kind="ExternalInput").ap()
    WqT = nc.dram_tensor("Wq8" if fp8 else "WqT", [DIM, DIM], FP, kind="ExternalInput").ap()
    WoT = nc.dram_tensor("WoT", [DIM, DIM], F16, kind="ExternalInput").ap()
    if fp8:
        maskCv = nc.dram_tensor("maskCv", [128, NC_T], F32, kind="ExternalInput").ap()
    cosC = nc.dram_tensor("cosC", [R, HD], F16, kind="ExternalInput").ap()
    sinC = nc.dram_tensor("sinC", [R, HD], F16, kind="ExternalInput").ap()
    cosF = nc.dram_tensor("cosF", [128, R], F16, kind="ExternalInput").ap()
    sinF = nc.dram_tensor("sinF", [128, R], F16, kind="ExternalInput").ap()
    maskC = nc.dram_tensor("maskC", [128, NC_T], F32, kind="ExternalInput").ap()
    ind16T = nc.dram_tensor("ind16T", [DIM, 16], F16, kind="ExternalInput").ap()
    ind16 = nc.dram_tensor("ind16", [16, DIM], F16, kind="ExternalInput").ap()
    Pmat = nc.dram_tensor("Pmat", [128, 128], F16, kind="ExternalInput").ap()

    # packed per-head diagonal Gram blocks: head 2p on partitions 0:64,
    # head 2p+1 on partitions 64:128, col block p*64
    kv_in_d = nc.dram_tensor("kv_in_d", [128, NPAIR * 64], F32)
    kv_out_d = nc.dram_tensor("kv_out_d", [128, NPAIR * 64], F32)

    out_d = nc.dram_tensor("out", [DIM, R], F16, kind="ExternalOutput").ap()
    # DCE-proof multi-rep timing: reps 0..n-2 write distinct scratch
    # tensors which are kept live by tiny reads at the end
    scratch = [
        nc.dram_tensor(f"scr{r}", [DIM, R], F16).ap() for r in range(reps - 1)
    ]

    def blkview(dram_ap, csl):
        return dram_ap.rearrange("(t p) c -> p t c", p=128)[:, :, csl]

    with tile.TileContext(nc) as tc:
        with ExitStack() as ctx:
            consts = ctx.enter_context(tc.tile_pool(name="consts", bufs=1))
            kvblk_pool = ctx.enter_context(tc.tile_pool(name="kvblk", bufs=1))

            cosC_t = consts.tile([128, NC_T * HD], F16, tag="cosC")
            sinC_t = consts.tile([128, NC_T * HD], F16, tag="sinC")
            nc.sync.dma_start(
                out=cosC_t[:].rearrange("p (t f) -> p t f", t=NC_T),
                in_=cosC[:].rearrange("(t p) f -> p t f", p=128),
            )
            nc.sync.dma_start(
                out=sinC_t[:].rearrange("p (t f) -> p t f", t=NC_T),
                in_=sinC[:].rearrange("(t p) f -> p t f", p=128),
            )
            maskC_t = consts.tile([128, NC_T], F32, tag="maskC")
            ind16T_t = consts.tile([128, NJ * 16], F16, tag="ind16T")
            ind16_t = consts.tile([16, DIM], F16, tag="ind16")
            P_t = consts.tile([128, 128], F16, tag="Pmat")
            nc.sync.dma_start(out=maskC_t[:], in_=maskC[:])
            if fp8:
                vmask_t = consts.tile([128, NC_T], F32, tag="maskCv")
                nc.sync.dma_start(out=vmask_t[:], in_=maskCv[:])
            else:
                vmask_t = maskC_t
            nc.sync.dma_start(
                out=ind16T_t[:].rearrange("p (t f) -> p t f", t=NJ),
                in_=ind16T[:].rearrange("(t p) f -> p t f", p=128),
            )
            nc.sync.dma_start(out=ind16_t[:], in_=ind16[:])
            nc.sync.dma_start(out=P_t[:], in_=Pmat[:])

            for _rep in range(reps):
              with ExitStack() as ctxX:
                xpool = ctxX.enter_context(tc.tile_pool(name="xpool", bufs=1))
                # x chunks on sync queue; phase-A weights lead the scalar
                # queue so k/v projections can start ASAP
                # x loaded in 4 column (row-range) slices so the first
                # c-tiles are ready after ~1MB instead of the full 4MB
                xT_all = xpool.tile([128, ND * R], FP, tag="xT")
                for xc in range(4):
                    csl = slice(xc * (R // 4), (xc + 1) * (R // 4))
                    nc.sync.dma_start(
                        out=xT_all[:].rearrange("p (t c) -> p t c", t=ND)[
                            :, :, csl
                        ],
                        in_=xT[:, csl].rearrange("(t p) c -> p t c", p=128),
                    )

                def xsl(dc, csl):
                    lo = dc * R
                    return xT_all[:, lo + csl.start : lo + csl.stop]

                # wBC pool created first (outlives ctxA; stack dealloc order)
                # but its loads are emitted AFTER wk/wv on the queues
                if "B" in phases:
                    wBC = ctxX.enter_context(tc.tile_pool(name="wBC", bufs=1))
                    wq_all = wBC.tile([128, ND * DIM], FP, tag="wq")
                    wo_all = wBC.tile([128, ND * DIM], F16, tag="wo")
                    cosF_t = wBC.tile([128, R], F16, tag="cosF")
                    sinF_t = wBC.tile([128, R], F16, tag="sinF")

                # phase-A weight pool scoped to ctxA (frees SBUF for B/C);
                # its loads lead the scalar queue so phase A starts ASAP
                ctxA = ctxX.enter_context(ExitStack())
                # wk/wv loaded in column halves, interleaved k-first, so the
                # first k-projection (needs wk cols 0:512) starts earliest
                wA = ctxA.enter_context(tc.tile_pool(name="wA", bufs=1))
                wk_all = wA.tile([128, ND * DIM], FP, tag="wk")
                wv_all = wA.tile([128, ND * DIM], FP, tag="wv")
                for half, wt, wsrc in (
                    (0, wk_all, WkT),
                    (0, wv_all, WvT),
                    (1, wk_all, WkT),
                    (1, wv_all, WvT),
                ):
                    jsl = slice(half * 512, (half + 1) * 512)
                    nc.scalar.dma_start(
                        out=wt[:].rearrange("p (t f) -> p t f", t=ND)[:, :, jsl],
                        in_=wsrc[:, jsl].rearrange("(t p) f -> p t f", p=128),
                    )

                if "B" in phases:
                    nc.sync.dma_start(
                        out=wq_all[:].rearrange("p (t f) -> p t f", t=ND),
                        in_=WqT[:].rearrange("(t p) f -> p t f", p=128),
                    )
                    nc.scalar.dma_start(
                        out=wo_all[:].rearrange("p (t f) -> p t f", t=ND),
                        in_=WoT[:].rearrange("(t p) f -> p t f", p=128),
                    )
                    nc.sync.dma_start(out=cosF_t[:], in_=cosF[:])
                    nc.scalar.dma_start(out=sinF_t[:], in_=sinF[:])

                # vkblk zeroed early (AR-independent)
                vkblk = kvblk_pool.tile([128, NPAIR * 128], F16, tag="vkblk")
                if "C" in phases:
                    nc.vector.memset(vkblk[:], 0.0)

                # ========= Phase A: k/v proj + process + vk Grams ==========
                if True:
                  if "A" in phases:
                    psA = ctxA.enter_context(
                        tc.tile_pool(name="psA", bufs=3, space="PSUM")
                    )
                    pskv = ctxA.enter_context(
                        tc.tile_pool(name="pskv", bufs=1, space="PSUM")
                    )
                    sbA = ctxA.enter_context(tc.tile_pool(name="sbA", bufs=2))
                    sb1 = ctxA.enter_context(tc.tile_pool(name="sb1", bufs=2))
                    smA = ctxA.enter_context(tc.tile_pool(name="smA", bufs=2))

                    vk_ps = pskv.tile([128, NPAIR * 128], F32, tag="kvps")
                    kv_pending = []

                    # vk Gram: vk[dv, dk] per head pair (cross blocks are
                    # garbage; only diag blocks get packed for the AR).
                    # On HW start=True zeroes the whole PSUM bank, so only
                    # the first matmul written to each bank may carry it.
                    def _emit_kv(item):
                        ct_, khat_, v_ = item
                        for p in range(NPAIR):
                            ps_ = slice(p * 128, (p + 1) * 128)
                            nc.tensor.matmul(
                                vk_ps[:, ps_],
                                v_[:, ps_],
                                khat_[:, ps_],
                                start=(
                                    True
                                    if sim_mode
                                    else (ct_ == 0 and p % 4 == 0)
                                ),
                                stop=(
                                    True if sim_mode else (ct_ == NC_T - 1)
                                ),
                            )

                    def x2(dc, csl):
                        # [128, 2, c] view over two adjacent d-chunks of x
                        return xT_all[:, dc * R : (dc + 2) * R].rearrange(
                            "p (t c) -> p t c", t=2
                        )[:, :, csl]

                    def w2(wt, dc, jsl):
                        return wt[:, dc * DIM : (dc + 2) * DIM].rearrange(
                            "p (t f) -> p t f", t=2
                        )[:, :, jsl]

                    for ct in range(NC_T):
                        cs = slice(ct * 128, (ct + 1) * 128)
                        k_ps = psA.tile([128, DIM], F32, tag="proj_ps")
                        v_ps = psA.tile([128, DIM], F32, tag="proj_ps")
                        for half in range(2):
                            js = slice(half * 512, (half + 1) * 512)
                            if fp8:
                                for dc in range(0, ND, 2):
                                    nc.tensor.matmul(
                                        k_ps[:, js],
                                        x2(dc, cs),
                                        w2(wk_all, dc, js),
                                        start=(dc == 0),
                                        stop=(dc == ND - 2),
                                        perf_mode=DoubleRow,
                                    )
                                for dc in range(0, ND, 2):
                                    nc.tensor.matmul(
                                        v_ps[:, js],
                                        x2(dc, cs),
                                        w2(wv_all, dc, js),
                                        start=(dc == 0),
                                        stop=(dc == ND - 2),
                                        perf_mode=DoubleRow,
                                    )
                            else:
                                for dc in range(ND):
                                    nc.tensor.matmul(
                                        k_ps[:, js],
                                        xsl(dc, cs),
                                        wk_all[
                                            :,
                                            dc * DIM + js.start : dc * DIM + js.stop,
                                        ],
                                        start=(dc == 0),
                                        stop=(dc == ND - 1),
                                    )
                                for dc in range(ND):
                                    nc.tensor.matmul(
                                        v_ps[:, js],
                                        xsl(dc, cs),
                                        wv_all[
                                            :,
                                            dc * DIM + js.start : dc * DIM + js.stop,
                                        ],
                                        start=(dc == 0),
                                        stop=(dc == ND - 1),
                                    )

                        # v: evict with mask fold (per-partition scale; also
                        # descales the fp8 weight upscale), cast
                        v_sb = sbA.tile([128, DIM], F16, tag="v_sb")
                        nc.scalar.activation(
                            v_sb[:],
                            v_ps[:],
                            Copy,
                            scale=vmask_t[:, ct : ct + 1],
                        )
                        # k: evict fast (cast fp16) to free the PSUM slot
                        k_sb = sbA.tile([128, DIM], F16, tag="k_sb")
                        nc.scalar.activation(k_sb[:], k_ps[:], Copy)
                        # squares for the l2 norm
                        sq = sbA.tile([128, DIM], F16, tag="sq")
                        nc.scalar.activation(sq[:], k_ps[:], Square)

                        cosb = (
                            cosC_t[:, ct * HD : (ct + 1) * HD]
                            .unsqueeze(1)
                            .broadcast_to([128, H, HD])
                        )
                        sinb4 = (
                            sinC_t[:, ct * HD : (ct + 1) * HD]
                            .rearrange("p (g two) -> p g two", two=2)
                            .unsqueeze(1)
                            .broadcast_to([128, H, HD // 2, 2])
                        )
                        k3 = k_sb[:].rearrange("p (h f) -> p h f", h=H)
                        k_sw = k_sb[:].rearrange(
                            "p (h g two) -> p h g two", h=H, two=2
                        )[:, :, :, ::-1]

                        m1 = sb1.tile([128, DIM], F16, tag="m1")
                        nc.vector.tensor_tensor(
                            m1[:].rearrange("p (h f) -> p h f", h=H), k3, cosb, MUL
                        )
                        red = smA.tile([128, H], F32, tag="red")
                        nc.vector.tensor_reduce(
                            red[:],
                            sq[:].rearrange("p (h f) -> p h f", h=H),
                            mybir.AxisListType.X,
                            ADD,
                        )
                        rcp = smA.tile([128, H], F32, tag="rcp")
                        nc.vector.reciprocal(rcp[:], red[:])
                        rs = smA.tile([128, H], F32, tag="rs")
                        nc.scalar.activation(rs[:], rcp[:], Sqrt)
                        rsm = smA.tile([128, H], F32, tag="rsm")
                        nc.vector.tensor_scalar_mul(
                            rsm[:], rs[:], maskC_t[:, ct : ct + 1]
                        )
                        m2 = sb1.tile([128, DIM], F16, tag="m2")
                        nc.vector.tensor_tensor(
                            m2[:].rearrange("p (h g two) -> p h g two", h=H, two=2),
                            k_sw,
                            sinb4,
                            MUL,
                        )
                        s = sb1.tile([128, DIM], F16, tag="s")
                        nc.vector.tensor_tensor(s[:], m1[:], m2[:], ADD)
                        khat = sbA.tile([128, DIM], F16, tag="khat")
                        rsb = rsm[:].unsqueeze(2).broadcast_to([128, H, HD])
                        nc.vector.tensor_tensor(
                            khat[:].rearrange("p (h f) -> p h f", h=H),
                            s[:].rearrange("p (h f) -> p h f", h=H),
                            rsb,
                            MUL,
                        )

                        # vk Grams are issued one iteration late (software
                        # pipelining) so PE never waits on the khat chain
                        kv_pending.append((ct, khat, v_sb))
                        if len(kv_pending) > 1:
                            _emit_kv(kv_pending.pop(0))

                    while kv_pending:
                        _emit_kv(kv_pending.pop(0))

                    # pack diag blocks [128, 512] and run the collective;
                    # the store trigger rides the DVE queue (right after the
                    # pack copies), the collective is on gpsimd
                    kv_pack = sbA.tile([128, NPAIR * 64], F32, tag="kv_pack")
                    nc.vector.tensor_copy(
                        kv_pack[0:64, :].rearrange("p (t f) -> p t f", t=NPAIR),
                        vk_ps[0:64, :].rearrange("p (t f) -> p t f", t=NPAIR)[
                            :, :, 0:64
                        ],
                    )
                    nc.vector.tensor_copy(
                        kv_pack[64:128, :].rearrange("p (t f) -> p t f", t=NPAIR),
                        vk_ps[64:128, :].rearrange("p (t f) -> p t f", t=NPAIR)[
                            :, :, 64:128
                        ],
                    )
                    nc.gpsimd.dma_start(out=kv_in_d.ap(), in_=kv_pack[:])
                    if sim_mode or no_ar:
                        # stand-in for the AllReduce so TimelineSim can run
                        nc.sync.dma_start(out=kv_out_d.ap(), in_=kv_in_d.ap())
                    else:
                        nc.gpsimd.collective_compute(
                            "AllReduce",
                            ADD,
                            replica_groups=[[0, 1, 2, 3], [4, 5, 6, 7]],
                            ins=[kv_in_d.ap().opt()],
                            outs=[kv_out_d.ap().opt()],
                        )

                ctxA.close()

                # kvp load fires the moment the AR completes (gpsimd queue);
                # nothing else is queued between the AR and this load
                if "C" in phases:
                    kvp = kvblk_pool.tile([128, NPAIR * 64], F32, tag="kvp")
                    nc.gpsimd.dma_start(out=kvp[:], in_=kv_out_d.ap())

                # ============ Phase B: q proj + l2norm + rope ==============
                with ExitStack() as ctxB:
                  if "B" in phases:
                    psB = ctxB.enter_context(
                        tc.tile_pool(name="psB", bufs=2, space="PSUM")
                    )
                    psN = ctxB.enter_context(
                        tc.tile_pool(name="psN", bufs=1, space="PSUM")
                    )
                    psO = ctxB.enter_context(
                        tc.tile_pool(name="psO", bufs=2, space="PSUM")
                    )
                    sbB = ctxB.enter_context(tc.tile_pool(name="sbB", bufs=3))
                    sbS = ctxB.enter_context(
                        tc.tile_pool(name="sbS", bufs=2 * NJ)
                    )
                    sbQ = ctxB.enter_context(tc.tile_pool(name="sbQ", bufs=NQ_T))
                    sbO = ctxB.enter_context(tc.tile_pool(name="sbO", bufs=3))

                    qh_tiles = []
                    for ct in range(NQ_T):
                        cs = slice(ct * 512, (ct + 1) * 512)
                        norms_ps = psN.tile([16, 512], F32, tag="norms")
                        qh_all = sbQ.tile(
                            [128, NJ * 512], F8 if fp8o else F16, tag="qhall"
                        )
                        q_sbs = []
                        sq_pending = []
                        # pass 1: projections + squares + norm accumulation
                        for jt in range(NJ):
                            jlo = jt * 128
                            q_ps = psB.tile([128, 512], F32, tag="q_ps")
                            if fp8:
                                for dc in range(0, ND, 2):
                                    nc.tensor.matmul(
                                        q_ps[:],
                                        w2(wq_all, dc, slice(jlo, jlo + 128)),
                                        x2(dc, cs),
                                        start=(dc == 0),
                                        stop=(dc == ND - 2),
                                        perf_mode=DoubleRow,
                                    )
                            else:
                                for dc in range(ND):
                                    nc.tensor.matmul(
                                        q_ps[:],
                                        wq_all[
                                            :, dc * DIM + jlo : dc * DIM + jlo + 128
                                        ],
                                        xsl(dc, cs),
                                        start=(dc == 0),
                                        stop=(dc == ND - 1),
                                    )
                            q_sb = sbS.tile([128, 512], F16, tag="q_sb")
                            nc.scalar.activation(q_sb[:], q_ps[:], Copy)
                            sq = sbB.tile([128, 512], F16, tag="sqB")
                            nc.scalar.activation(sq[:], q_ps[:], Square)
                            # norms matmul issued one jt late so PE never
                            # waits on the act-engine square
                            sq_pending.append((jt, sq))
                            if len(sq_pending) > 1:
                                jt_, sq_ = sq_pending.pop(0)
                                nc.tensor.matmul(
                                    norms_ps[:],
                                    ind16T_t[:, jt_ * 16 : (jt_ + 1) * 16],
                                    sq_[:],
                                    start=(jt_ == 0),
                                    stop=False,
                                )
                            q_sbs.append(q_sb)

                        jt_, sq_ = sq_pending.pop(0)
                        nc.tensor.matmul(
                            norms_ps[:],
                            ind16T_t[:, jt_ * 16 : (jt_ + 1) * 16],
                            sq_[:],
                            start=False,
                            stop=True,
                        )

                        rcpB = sbB.tile([16, 512], F32, tag="rcpB")
                        nc.vector.reciprocal(rcpB[:], norms_ps[:])
                        rs16 = sbB.tile([16, 512], F16, tag="rs16")
                        # fp8o: qhat carries an extra XS_Q upscale via rs16
                        nc.scalar.activation(
                            rs16[:],
                            rcpB[:],
                            Sqrt,
                            scale=XS_Q * XS_Q if fp8o else 1.0,
                        )

                        # pass 2: rotation + rope + scale into qh_all
                        for jt in range(NJ):
                            q_sb = q_sbs[jt]
                            rot_ps = psB.tile([128, 512], F32, tag="rotrep")
                            nc.tensor.matmul(
                                rot_ps[:], P_t[:], q_sb[:], start=True, stop=True
                            )
                            rep_ps = psB.tile([128, 512], F32, tag="rotrep")
                            nc.tensor.matmul(
                                rep_ps[:],
                                ind16_t[:, jt * 128 : (jt + 1) * 128],
                                rs16[:],
                                start=True,
                                stop=True,
                            )
                            # t1 rides gpsimd (idle during phase B; its queue
                            # position is after the AR trigger but these do
                            # not depend on the AR, so no stall)
                            t1 = sbB.tile([128, 512], F16, tag="t1")
                            nc.gpsimd.tensor_tensor(
                                t1[:], q_sb[:], cosF_t[:, cs], MUL
                            )
                            t2 = sbB.tile([128, 512], F16, tag="t2")
                            nc.vector.tensor_tensor(
                                t2[:], rot_ps[:], sinF_t[:, cs], MUL
                            )
                            s = sbB.tile([128, 512], F16, tag="sB")
                            nc.vector.tensor_tensor(s[:], t1[:], t2[:], ADD)
                            nc.vector.tensor_tensor(
                                qh_all[:, jt * 512 : (jt + 1) * 512],
                                s[:],
                                rep_ps[:],
                                MUL,
                            )
                        qh_tiles.append((cs, qh_all))

                    # ====== AR-dependent: blockdiag unpack + G = vk@WoT =====
                    # the unpack copies ride the DVE queue at B-end, when the
                    # AR (and kvp load) have long completed
                  if "C" in phases and "B" in phases:
                    nc.vector.tensor_copy(
                        vkblk[0:64, :].rearrange("p (t f) -> p t f", t=NPAIR)[
                            :, :, 0:64
                        ],
                        kvp[0:64, :].rearrange("p (t f) -> p t f", t=NPAIR),
                    )
                    nc.vector.tensor_copy(
                        vkblk[64:128, :].rearrange("p (t f) -> p t f", t=NPAIR)[
                            :, :, 64:128
                        ],
                        kvp[64:128, :].rearrange("p (t f) -> p t f", t=NPAIR),
                    )

                    G_sb = kvblk_pool.tile(
                        [128, NPAIR * DIM], F8 if fp8o else F16, tag="G_sb"
                    )
                    for p in range(NPAIR):
                        for half in range(2):
                            g_ps = psO.tile([128, 512], F32, tag="o_ps")
                            nc.tensor.matmul(
                                g_ps[:],
                                vkblk[:, p * 128 : (p + 1) * 128],
                                wo_all[
                                    :,
                                    p * DIM
                                    + half * 512 : p * DIM
                                    + half * 512
                                    + 512,
                                ],
                                start=True,
                                stop=True,
                            )
                            gdst = G_sb[
                                :,
                                p * DIM + half * 512 : p * DIM + half * 512 + 512,
                            ]
                            nc.scalar.activation(
                                gdst, g_ps[:], Copy, scale=XS_G if fp8o else 1.0
                            )

                    # ============= Phase C: out = G^T q, store ==============
                    out_tgt = scratch[_rep] if _rep < reps - 1 else out_d
                    for cs, qh_all in qh_tiles:
                        for et in range(NJ):
                            elo = et * 128
                            o_ps = psO.tile([128, 512], F32, tag="o_ps")
                            if fp8o:
                                for p in range(0, NPAIR, 2):
                                    nc.tensor.matmul(
                                        o_ps[:],
                                        G_sb[
                                            :, p * DIM : (p + 2) * DIM
                                        ].rearrange("p (t f) -> p t f", t=2)[
                                            :, :, elo : elo + 128
                                        ],
                                        qh_all[
                                            :, p * 512 : (p + 2) * 512
                                        ].rearrange("p (t c) -> p t c", t=2),
                                        start=(p == 0),
                                        stop=(p == NPAIR - 2),
                                        perf_mode=DoubleRow,
                                    )
                            else:
                                for p in range(NPAIR):
                                    nc.tensor.matmul(
                                        o_ps[:],
                                        G_sb[
                                            :, p * DIM + elo : p * DIM + elo + 128
                                        ],
                                        qh_all[:, p * 512 : (p + 1) * 512],
                                        start=(p == 0),
                                        stop=(p == NPAIR - 1),
                                    )
                            o_sb = sbO.tile([128, 512], F16, tag="o_sb")
                            nc.scalar.activation(
                                o_sb[:],
                                o_ps[:],
                                Copy,
                                scale=1.0 / (XS_G * XS_Q) if fp8o else 1.0,
                            )
                            # store each 128-row feature chunk as it's ready
                            nc.scalar.dma_start(
                                out=out_tgt[elo : elo + 128, cs],
                                in_=o_sb[:],
                            )

            # keep-alive reads for the scratch outputs (timing builds only)
            for r, scr in enumerate(scratch):
                ka = consts.tile([16, 16], F16, tag=f"ka{r}")
                nc.sync.dma_start(out=ka[:], in_=scr[0:16, 0:16])
                nc.sync.dma_start(
                    out=out_d[0:16, r * 16 : (r + 1) * 16], in_=ka[:]
                )

    nc.compile()
    return nc


_NC_CACHE = None


def _get_nc():
    global _NC_CACHE
    if _NC_CACHE is None:
        _NC_CACHE = build_nc()
    return _NC_CACHE


def make_in_maps(x, mask, Wq, Wk, Wv, Wo, norm_const):
    x = np.asarray(x, np.float32)
    mask = np.asarray(mask)
    Wq = np.asarray(Wq, np.float32)
    Wk = np.asarray(Wk, np.float32)
    Wv = np.asarray(Wv, np.float32)
    Wo = np.asarray(Wo, np.float32)
    norm_const = np.asarray(norm_const, np.float32).reshape(H)

    sig = 1.0 / (1.0 + np.exp(-norm_const.astype(np.float64)))
    svec = np.float64(C) ** (-sig)  # [H]
    s_cols = np.repeat(svec, HD)  # [DIM]

    f16 = np.float16
    f8 = mybir.dt.np(F8)
    WvTf = np.ascontiguousarray((Wv * s_cols[:, None].astype(np.float32)).T)
    WkT = np.ascontiguousarray(Wk.T).astype(f16)
    WvT = WvTf.astype(f16)
    WqT = np.ascontiguousarray(Wq.T).astype(f16)
    WoT = np.ascontiguousarray(Wo.T).astype(f16)
    Wk8 = (Wk.T * XS_K).astype(f8)
    Wq8 = (Wq.T * XS_K).astype(f8)
    Wv8 = (WvTf * XS_V).astype(f8)

    inv_freq = 1.0 / (
        ROPE_THETA ** (np.arange(0, HD, 2, dtype=np.float64) / HD)
    )  # [32]
    freq_of_j = np.repeat(inv_freq, 2)  # [64] interleaved

    ind16T = np.zeros((DIM, 16), f16)
    for jt in range(NJ):
        for kk in range(128):
            ind16T[jt * 128 + kk, 2 * jt + (kk >= 64)] = 1.0

    ind16 = np.zeros((16, DIM), f16)
    for jt in range(NJ):
        for m in range(128):
            ind16[2 * jt + (m >= 64), jt * 128 + m] = 1.0

    Pmat = np.zeros((128, 128), f16)
    for i in range(64):
        Pmat[2 * i + 1, 2 * i] = -1.0  # out[2i] = -q[2i+1]
        Pmat[2 * i, 2 * i + 1] = 1.0  # out[2i+1] = q[2i]

    in_maps = []
    for core in range(N_CORES):
        b = core // (N_CORES // B)
        cc = core % (N_CORES // B)
        c0 = cc * R
        pos = (c0 + np.arange(R)).astype(np.float64)

        xTc = np.ascontiguousarray(x[b, c0 : c0 + R, :].T).astype(f16)

        angC = pos[:, None] * freq_of_j[None, :]  # [R, 64]
        cosCc = np.cos(angC).astype(f16)
        sinCc = np.sin(angC).astype(np.float32)
        # sign fold for the swap formulation: even j -> -sin, odd j -> +sin
        sinCc[:, 0::2] *= -1.0
        sinCc = sinCc.astype(f16)

        angF = freq_of_j[:, None] * pos[None, :]  # [64, R]
        angF2 = np.concatenate([angF, angF], axis=0)  # [128, R]
        cosFc = np.cos(angF2).astype(f16)
        sinFc = np.sin(angF2).astype(f16)

        mrow = mask[b, c0 : c0 + R].astype(np.float32)  # [R]
        maskCc = np.ascontiguousarray(mrow.reshape(NC_T, 128).T)  # [128, NC_T]

        in_maps.append(
            {
                "xT": xTc,
                "xT8": xTc.astype(f8),
                "WkT": WkT,
                "WvT": WvT,
                "WqT": WqT,
                "WoT": WoT,
                "Wk8": Wk8,
                "Wq8": Wq8,
                "Wv8": Wv8,
                "maskCv": maskCc / XS_V,
                "cosC": cosCc,
                "sinC": sinCc,
                "cosF": cosFc,
                "sinF": sinFc,
                "maskC": maskCc,
                "ind16T": ind16T,
                "ind16": ind16,
                "Pmat": Pmat,
            }
        )
    return in_maps


def assemble_output(results, mask):
    out = np.empty((B, C, DIM), np.float32)
    for core in range(N_CORES):
        b = core // (N_CORES // B)
        cc = core % (N_CORES // B)
        c0 = cc * R
        out[b, c0 : c0 + R, :] = results[core]["out"].T.astype(np.float32)
    # q-side mask: masked rows produce zero output
    out *= np.asarray(mask)[:, :, None].astype(np.float32)
    return out


def kernel(x, mask, Wq, Wk, Wv, Wo, norm_const):
    nc = _get_nc()
    in_maps = make_in_maps(x, mask, Wq, Wk, Wv, Wo, norm_const)
    res = run_bass_kernel_spmd(nc, in_maps, list(range(N_CORES)))
    return assemble_output(res.results, mask)


# revision 68
# speedup vs baseline: 5.1829x; 1.4647x over previous
"""Trainium2 Bass kernel for nn_Attention_43413529428606 (linear attention
with l2-normed q/k, interleaved RoPE, mask, per-head power scaling).

Sharding: the 16384 (batch*seq) rows are split across 8 NeuronCores, 2048
rows each; cores 0-3 take batch 0, cores 4-7 batch 1.  Each core computes
k/v projections for its rows (all 16 heads), applies l2norm+RoPE+mask,
accumulates the per-head v^T k Gram, AllReduces the packed diagonal blocks
(128 KB fp16) within its batch group, folds Wo into the reduced Gram
(G = blockdiag(vk) @ Wo^T), and finishes with out = G^T q — the attention
and output projection collapse into one GEMM.  The q-side phase (proj +
l2norm + RoPE) is emitted before anything that depends on the AllReduce and
the AllReduce-dependent load rides the gpsimd queue, so the collective
overlaps the whole q phase instead of stalling the engines.  The data path
is fp16 (fp32 PSUM accumulation); only {Copy, Square, Sqrt} activation
functions are used so a single act table load suffices (norm rsqrt =
DVE reciprocal + act Sqrt).  The q-side mask is applied host-side on the
output rows.

Self-contained: hardcodes all shapes; no sibling imports.
"""

import sys

for _p in ("/opt/trn_rl_repo",):
    if _p not in sys.path:
        sys.path.append(_p)

from contextlib import ExitStack

import numpy as np

import concourse.bass as bass
import concourse.bacc as bacc
import concourse.tile as tile
from concourse import mybir
from concourse.bass_utils import run_bass_kernel_spmd

F32 = mybir.dt.float32
F16 = mybir.dt.float16
F8 = mybir.dt.float8e4
DoubleRow = mybir.MatmulPerfMode.DoubleRow

# power-of-two pre-scales for the fp8 data path (exactly invertible)
XS_K = 32.0  # Wk/Wq host upscale; cancels in the l2 norm
XS_V = 8192.0  # Wv host upscale; descaled in the v eviction
XS_G = 64.0  # G eviction upscale into fp8
XS_Q = 8.0  # qhat upscale into fp8 (folded into rs16)

DIM = 1024
H = 16
HD = 64
B = 2
C = 8192
ROPE_THETA = 10000.0

N_CORES = 8
R = (B * C) // N_CORES  # 2048 rows per core
NC_T = R // 128  # 16 c-tiles of 128 (phase A)
NQ_T = R // 512  # 4 c-supertiles of 512 (q and out phases)
ND = DIM // 128  # 8 d-chunks
NJ = DIM // 128  # 8 j-tiles
NPAIR = H // 2  # 8 head pairs

Copy = mybir.ActivationFunctionType.Copy
Square = mybir.ActivationFunctionType.Square
Sqrt = mybir.ActivationFunctionType.Sqrt
MUL = mybir.AluOpType.mult
ADD = mybir.AluOpType.add


def build_nc(
    sim_mode=False,
    phases="ABC",
    reps=1,
    no_ar=False,
    fp8=False,
    fp8o=False,
    big_store=True,
    ar16=True,
):
    nc = bacc.Bacc(
        "TRN2",
        target_bir_lowering=False,
        debug=False,
        num_devices=1 if sim_mode else N_CORES,
    )

    # ---- DRAM parameters (per-core shapes, fp8/fp16 data path) ----
    FP = F8 if fp8 else F16
    xT = nc.dram_tensor("xT8" if fp8 else "xT", [DIM, R], FP, kind="ExternalInput").ap()
    WkT = nc.dram_tensor("Wk8" if fp8 else "WkT", [DIM, DIM], FP, kind="ExternalInput").ap()
    WvT = nc.dram_tensor("Wv8" if fp8 else "WvT", [DIM, DIM], FP, kind="ExternalInput").ap()
    WqT = nc.dram_tensor("Wq8" if fp8 else "WqT", [DIM, DIM], FP, kind="ExternalInput").ap()
    WoT = nc.dram_tensor("WoT", [DIM, DIM], F16, kind="ExternalInput").ap()
    if fp8:
        maskCv = nc.dram_tensor("maskCv", [128, NC_T], F32, kind="ExternalInput").ap()
    cosC = nc.dram_tensor("cosC", [R, HD], F16, kind="ExternalInput").ap()
    sinC = nc.dram_tensor("sinC", [R, HD], F16, kind="ExternalInput").ap()
    cosF = nc.dram_tensor("cosF", [128, R], F16, kind="ExternalInput").ap()
    sinF = nc.dram_tensor("sinF", [128, R], F16, kind="ExternalInput").ap()
    maskC = nc.dram_tensor("maskC", [128, NC_T], F32, kind="ExternalInput").ap()
    ind16T = nc.dram_tensor("ind16T", [DIM, 16], F16, kind="ExternalInput").ap()
    ind16 = nc.dram_tensor("ind16", [16, DIM], F16, kind="ExternalInput").ap()
    Pmat = nc.dram_tensor("Pmat", [128, 128], F16, kind="ExternalInput").ap()

    # packed per-head diagonal Gram blocks: head 2p on partitions 0:64,
    # head 2p+1 on partitions 64:128, col block p*64
    FAR = F16 if ar16 else F32
    kv_in_d = nc.dram_tensor("kv_in_d", [128, NPAIR * 64], FAR)
    kv_out_d = nc.dram_tensor("kv_out_d", [128, NPAIR * 64], FAR)

    out_d = nc.dram_tensor("out", [DIM, R], F16, kind="ExternalOutput").ap()
    # DCE-proof multi-rep timing: reps 0..n-2 write distinct scratch
    # tensors which are kept live by tiny reads at the end
    scratch = [
        nc.dram_tensor(f"scr{r}", [DIM, R], F16).ap() for r in range(reps - 1)
    ]

    def blkview(dram_ap, csl):
        return dram_ap.rearrange("(t p) c -> p t c", p=128)[:, :, csl]

    with tile.TileContext(nc) as tc:
        with ExitStack() as ctx:
            consts = ctx.enter_context(tc.tile_pool(name="consts", bufs=1))
            kvblk_pool = ctx.enter_context(tc.tile_pool(name="kvblk", bufs=1))

            # consts tiles created here; all but maskC are LOADED after the
            # first x slice (see below) to keep the lead-in critical path
            # to [maskC, x0] + [wk h0]
            cosC_t = consts.tile([128, NC_T * HD], F16, tag="cosC")
            sinC_t = consts.tile([128, NC_T * HD], F16, tag="sinC")
            maskC_t = consts.tile([128, NC_T], F32, tag="maskC")
            ind16T_t = consts.tile([128, NJ * 16], F16, tag="ind16T")
            ind16_t = consts.tile([16, DIM], F16, tag="ind16")
            P_t = consts.tile([128, 128], F16, tag="Pmat")
            nc.sync.dma_start(out=maskC_t[:], in_=maskC[:])
            if fp8:
                vmask_t = consts.tile([128, NC_T], F32, tag="maskCv")
                nc.sync.dma_start(out=vmask_t[:], in_=maskCv[:])
            else:
                vmask_t = maskC_t

            def load_consts():
                nc.sync.dma_start(
                    out=cosC_t[:].rearrange("p (t f) -> p t f", t=NC_T),
                    in_=cosC[:].rearrange("(t p) f -> p t f", p=128),
                )
                nc.sync.dma_start(
                    out=sinC_t[:].rearrange("p (t f) -> p t f", t=NC_T),
                    in_=sinC[:].rearrange("(t p) f -> p t f", p=128),
                )
                nc.sync.dma_start(
                    out=ind16T_t[:].rearrange("p (t f) -> p t f", t=NJ),
                    in_=ind16T[:].rearrange("(t p) f -> p t f", p=128),
                )
                nc.sync.dma_start(out=ind16_t[:], in_=ind16[:])
                nc.sync.dma_start(out=P_t[:], in_=Pmat[:])

            for _rep in range(reps):
              with ExitStack() as ctxX:
                xpool = ctxX.enter_context(tc.tile_pool(name="xpool", bufs=1))
                # x chunks on sync queue; phase-A weights lead the scalar
                # queue so k/v projections can start ASAP
                # x loaded in 4 column (row-range) slices so the first
                # c-tiles are ready after ~1MB instead of the full 4MB
                xT_all = xpool.tile([128, ND * R], FP, tag="xT")
                for xc in range(4):
                    csl = slice(xc * (R // 4), (xc + 1) * (R // 4))
                    nc.sync.dma_start(
                        out=xT_all[:].rearrange("p (t c) -> p t c", t=ND)[
                            :, :, csl
                        ],
                        in_=xT[:, csl].rearrange("(t p) c -> p t c", p=128),
                    )
                    if xc == 0 and _rep == 0:
                        load_consts()

                def xsl(dc, csl):
                    lo = dc * R
                    return xT_all[:, lo + csl.start : lo + csl.stop]

                # wBC pool created first (outlives ctxA; stack dealloc order)
                # but its loads are emitted AFTER wk/wv on the queues
                if "B" in phases:
                    wBC = ctxX.enter_context(tc.tile_pool(name="wBC", bufs=1))
                    wq_all = wBC.tile([128, ND * DIM], FP, tag="wq")
                    wo_all = wBC.tile([128, ND * DIM], F16, tag="wo")
                    cosF_t = wBC.tile([128, R], F16, tag="cosF")
                    sinF_t = wBC.tile([128, R], F16, tag="sinF")

                # phase-A weight pool scoped to ctxA (frees SBUF for B/C);
                # its loads lead the scalar queue so phase A starts ASAP
                ctxA = ctxX.enter_context(ExitStack())
                # wk/wv loaded in column halves, interleaved k-first, so the
                # first k-projection (needs wk cols 0:512) starts earliest
                wA = ctxA.enter_context(tc.tile_pool(name="wA", bufs=1))
                wk_all = wA.tile([128, ND * DIM], FP, tag="wk")
                wv_all = wA.tile([128, ND * DIM], FP, tag="wv")
                for half, wt, wsrc in (
                    (0, wk_all, WkT),
                    (0, wv_all, WvT),
                    (1, wk_all, WkT),
                    (1, wv_all, WvT),
                ):
                    jsl = slice(half * 512, (half + 1) * 512)
                    nc.scalar.dma_start(
                        out=wt[:].rearrange("p (t f) -> p t f", t=ND)[:, :, jsl],
                        in_=wsrc[:, jsl].rearrange("(t p) f -> p t f", p=128),
                    )

                if "B" in phases:
                    nc.sync.dma_start(
                        out=wq_all[:].rearrange("p (t f) -> p t f", t=ND),
                        in_=WqT[:].rearrange("(t p) f -> p t f", p=128),
                    )
                    nc.scalar.dma_start(
                        out=wo_all[:].rearrange("p (t f) -> p t f", t=ND),
                        in_=WoT[:].rearrange("(t p) f -> p t f", p=128),
                    )
                    nc.sync.dma_start(out=cosF_t[:], in_=cosF[:])
                    nc.scalar.dma_start(out=sinF_t[:], in_=sinF[:])

                # vkblk zeroed early (AR-independent)
                vkblk = kvblk_pool.tile([128, NPAIR * 128], F16, tag="vkblk")
                if "C" in phases:
                    nc.vector.memset(vkblk[:], 0.0)

                # ========= Phase A: k/v proj + process + vk Grams ==========
                if True:
                  if "A" in phases:
                    psA = ctxA.enter_context(
                        tc.tile_pool(name="psA", bufs=3, space="PSUM")
                    )
                    pskv = ctxA.enter_context(
                        tc.tile_pool(name="pskv", bufs=1, space="PSUM")
                    )
                    sbA = ctxA.enter_context(tc.tile_pool(name="sbA", bufs=2))
                    sb1 = ctxA.enter_context(tc.tile_pool(name="sb1", bufs=2))
                    smA = ctxA.enter_context(tc.tile_pool(name="smA", bufs=2))

                    vk_ps = pskv.tile([128, NPAIR * 128], F32, tag="kvps")
                    kv_pending = []

                    # vk Gram: vk[dv, dk] per head pair (cross blocks are
                    # garbage; only diag blocks get packed for the AR).
                    # On HW start=True zeroes the whole PSUM bank, so only
                    # the first matmul written to each bank may carry it.
                    def _emit_kv(item):
                        ct_, khat_, v_ = item
                        for p in range(NPAIR):
                            ps_ = slice(p * 128, (p + 1) * 128)
                            nc.tensor.matmul(
                                vk_ps[:, ps_],
                                v_[:, ps_],
                                khat_[:, ps_],
                                start=(
                                    True
                                    if sim_mode
                                    else (ct_ == 0 and p % 4 == 0)
                                ),
                                stop=(
                                    True if sim_mode else (ct_ == NC_T - 1)
                                ),
                            )

                    def x2(dc, csl):
                        # [128, 2, c] view over two adjacent d-chunks of x
                        return xT_all[:, dc * R : (dc + 2) * R].rearrange(
                            "p (t c) -> p t c", t=2
                        )[:, :, csl]

                    def w2(wt, dc, jsl):
                        return wt[:, dc * DIM : (dc + 2) * DIM].rearrange(
                            "p (t f) -> p t f", t=2
                        )[:, :, jsl]

                    for ct in range(NC_T):
                        cs = slice(ct * 128, (ct + 1) * 128)
                        k_ps = psA.tile([128, DIM], F32, tag="proj_ps")
                        v_ps = psA.tile([128, DIM], F32, tag="proj_ps")
                        for half in range(2):
                            js = slice(half * 512, (half + 1) * 512)
                            if fp8:
                                for dc in range(0, ND, 2):
                                    nc.tensor.matmul(
                                        k_ps[:, js],
                                        x2(dc, cs),
                                        w2(wk_all, dc, js),
                                        start=(dc == 0),
                                        stop=(dc == ND - 2),
                                        perf_mode=DoubleRow,
                                    )
                                for dc in range(0, ND, 2):
                                    nc.tensor.matmul(
                                        v_ps[:, js],
                                        x2(dc, cs),
                                        w2(wv_all, dc, js),
                                        start=(dc == 0),
                                        stop=(dc == ND - 2),
                                        perf_mode=DoubleRow,
                                    )
                            else:
                                for dc in range(ND):
                                    nc.tensor.matmul(
                                        k_ps[:, js],
                                        xsl(dc, cs),
                                        wk_all[
                                            :,
                                            dc * DIM + js.start : dc * DIM + js.stop,
                                        ],
                                        start=(dc == 0),
                                        stop=(dc == ND - 1),
                                    )
                                for dc in range(ND):
                                    nc.tensor.matmul(
                                        v_ps[:, js],
                                        xsl(dc, cs),
                                        wv_all[
                                            :,
                                            dc * DIM + js.start : dc * DIM + js.stop,
                                        ],
                                        start=(dc == 0),
                                        stop=(dc == ND - 1),
                                    )

                        # v: evict with mask fold (per-partition scale; also
                        # descales the fp8 weight upscale), cast
                        v_sb = sbA.tile([128, DIM], F16, tag="v_sb")
                        nc.scalar.activation(
                            v_sb[:],
                            v_ps[:],
                            Copy,
                            scale=vmask_t[:, ct : ct + 1],
                        )
                        # k: evict fast (cast fp16) to free the PSUM slot
                        k_sb = sbA.tile([128, DIM], F16, tag="k_sb")
                        nc.scalar.activation(k_sb[:], k_ps[:], Copy)
                        # squares for the l2 norm
                        sq = sbA.tile([128, DIM], F16, tag="sq")
                        nc.scalar.activation(sq[:], k_ps[:], Square)

                        cosb = (
                            cosC_t[:, ct * HD : (ct + 1) * HD]
                            .unsqueeze(1)
                            .broadcast_to([128, H, HD])
                        )
                        sinb4 = (
                            sinC_t[:, ct * HD : (ct + 1) * HD]
                            .rearrange("p (g two) -> p g two", two=2)
                            .unsqueeze(1)
                            .broadcast_to([128, H, HD // 2, 2])
                        )
                        k3 = k_sb[:].rearrange("p (h f) -> p h f", h=H)
                        k_sw = k_sb[:].rearrange(
                            "p (h g two) -> p h g two", h=H, two=2
                        )[:, :, :, ::-1]

                        m1 = sb1.tile([128, DIM], F16, tag="m1")
                        nc.vector.tensor_tensor(
                            m1[:].rearrange("p (h f) -> p h f", h=H), k3, cosb, MUL
                        )
                        red = smA.tile([128, H], F32, tag="red")
                        nc.vector.tensor_reduce(
                            red[:],
                            sq[:].rearrange("p (h f) -> p h f", h=H),
                            mybir.AxisListType.X,
                            ADD,
                        )
                        rcp = smA.tile([128, H], F32, tag="rcp")
                        nc.vector.reciprocal(rcp[:], red[:])
                        rs = smA.tile([128, H], F32, tag="rs")
                        nc.scalar.activation(rs[:], rcp[:], Sqrt)
                        rsm = smA.tile([128, H], F32, tag="rsm")
                        nc.vector.tensor_scalar_mul(
                            rsm[:], rs[:], maskC_t[:, ct : ct + 1]
                        )
                        m2 = sb1.tile([128, DIM], F16, tag="m2")
                        nc.vector.tensor_tensor(
                            m2[:].rearrange("p (h g two) -> p h g two", h=H, two=2),
                            k_sw,
                            sinb4,
                            MUL,
                        )
                        s = sb1.tile([128, DIM], F16, tag="s")
                        nc.vector.tensor_tensor(s[:], m1[:], m2[:], ADD)
                        khat = sbA.tile([128, DIM], F16, tag="khat")
                        rsb = rsm[:].unsqueeze(2).broadcast_to([128, H, HD])
                        nc.vector.tensor_tensor(
                            khat[:].rearrange("p (h f) -> p h f", h=H),
                            s[:].rearrange("p (h f) -> p h f", h=H),
                            rsb,
                            MUL,
                        )

                        # vk Grams are issued one iteration late (software
                        # pipelining) so PE never waits on the khat chain
                        kv_pending.append((ct, khat, v_sb))
                        if len(kv_pending) > 1:
                            _emit_kv(kv_pending.pop(0))

                    while kv_pending:
                        _emit_kv(kv_pending.pop(0))

                    # pack diag blocks [128, 512] and run the collective;
                    # the store trigger rides the DVE queue (right after the
                    # pack copies), the collective is on gpsimd
                    kv_pack = sbA.tile([128, NPAIR * 64], FAR, tag="kv_pack")
                    nc.vector.tensor_copy(
                        kv_pack[0:64, :].rearrange("p (t f) -> p t f", t=NPAIR),
                        vk_ps[0:64, :].rearrange("p (t f) -> p t f", t=NPAIR)[
                            :, :, 0:64
                        ],
                    )
                    nc.vector.tensor_copy(
                        kv_pack[64:128, :].rearrange("p (t f) -> p t f", t=NPAIR),
                        vk_ps[64:128, :].rearrange("p (t f) -> p t f", t=NPAIR)[
                            :, :, 64:128
                        ],
                    )
                    nc.gpsimd.dma_start(out=kv_in_d.ap(), in_=kv_pack[:])
                    if sim_mode or no_ar:
                        # stand-in for the AllReduce so TimelineSim can run
                        nc.sync.dma_start(out=kv_out_d.ap(), in_=kv_in_d.ap())
                    else:
                        nc.gpsimd.collective_compute(
                            "AllReduce",
                            ADD,
                            replica_groups=[[0, 1, 2, 3], [4, 5, 6, 7]],
                            ins=[kv_in_d.ap().opt()],
                            outs=[kv_out_d.ap().opt()],
                        )

                ctxA.close()

                # kvp load fires the moment the AR completes (gpsimd queue);
                # nothing else is queued between the AR and this load
                if "C" in phases:
                    kvp = kvblk_pool.tile([128, NPAIR * 64], FAR, tag="kvp")
                    nc.gpsimd.dma_start(out=kvp[:], in_=kv_out_d.ap())

                # ============ Phase B: q proj + l2norm + rope ==============
                with ExitStack() as ctxB:
                  if "B" in phases:
                    psQ = ctxB.enter_context(
                        tc.tile_pool(name="psQ", bufs=3, space="PSUM")
                    )
                    psB = ctxB.enter_context(
                        tc.tile_pool(name="psB", bufs=2, space="PSUM")
                    )
                    psN = ctxB.enter_context(
                        tc.tile_pool(name="psN", bufs=1, space="PSUM")
                    )
                    psO = ctxB.enter_context(
                        tc.tile_pool(name="psO", bufs=2, space="PSUM")
                    )
                    sbB = ctxB.enter_context(tc.tile_pool(name="sbB", bufs=3))
                    sbS = ctxB.enter_context(
                        tc.tile_pool(name="sbS", bufs=2 * NJ)
                    )
                    sbQ = ctxB.enter_context(tc.tile_pool(name="sbQ", bufs=NQ_T))
                    sbO = ctxB.enter_context(
                        tc.tile_pool(name="sbO", bufs=2 if big_store else 3)
                    )

                    qh_tiles = []
                    for ct in range(NQ_T):
                        cs = slice(ct * 512, (ct + 1) * 512)
                        norms_ps = psN.tile([16, 512], F32, tag="norms")
                        qh_all = sbQ.tile(
                            [128, NJ * 512], F8 if fp8o else F16, tag="qhall"
                        )
                        q_sbs = []
                        sq_pending = []
                        # pass 1: projections + squares + norm accumulation
                        for jt in range(NJ):
                            jlo = jt * 128
                            q_ps = psQ.tile([128, 512], F32, tag="q_ps")
                            if fp8:
                                for dc in range(0, ND, 2):
                                    nc.tensor.matmul(
                                        q_ps[:],
                                        w2(wq_all, dc, slice(jlo, jlo + 128)),
                                        x2(dc, cs),
                                        start=(dc == 0),
                                        stop=(dc == ND - 2),
                                        perf_mode=DoubleRow,
                                    )
                            else:
                                for dc in range(ND):
                                    nc.tensor.matmul(
                                        q_ps[:],
                                        wq_all[
                                            :, dc * DIM + jlo : dc * DIM + jlo + 128
                                        ],
                                        xsl(dc, cs),
                                        start=(dc == 0),
                                        stop=(dc == ND - 1),
                                    )
                            q_sb = sbS.tile([128, 512], F16, tag="q_sb")
                            nc.scalar.activation(q_sb[:], q_ps[:], Copy)
                            sq = sbB.tile([128, 512], F16, tag="sqB")
                            nc.scalar.activation(sq[:], q_ps[:], Square)
                            # norms matmul issued one jt late so PE never
                            # waits on the act-engine square
                            sq_pending.append((jt, sq))
                            if len(sq_pending) > 1:
                                jt_, sq_ = sq_pending.pop(0)
                                nc.tensor.matmul(
                                    norms_ps[:],
                                    ind16T_t[:, jt_ * 16 : (jt_ + 1) * 16],
                                    sq_[:],
                                    start=(jt_ == 0),
                                    stop=False,
                                )
                            q_sbs.append(q_sb)

                        jt_, sq_ = sq_pending.pop(0)
                        nc.tensor.matmul(
                            norms_ps[:],
                            ind16T_t[:, jt_ * 16 : (jt_ + 1) * 16],
                            sq_[:],
                            start=False,
                            stop=True,
                        )

                        rcpB = sbB.tile([16, 512], F32, tag="rcpB")
                        nc.vector.reciprocal(rcpB[:], norms_ps[:])
                        rs16 = sbB.tile([16, 512], F16, tag="rs16")
                        # fp8o: qhat carries an extra XS_Q upscale via rs16
                        nc.scalar.activation(
                            rs16[:],
                            rcpB[:],
                            Sqrt,
                            scale=XS_Q * XS_Q if fp8o else 1.0,
                        )

                        # pass 2: rotation + rope + scale into qh_all
                        for jt in range(NJ):
                            q_sb = q_sbs[jt]
                            rot_ps = psB.tile([128, 512], F32, tag="rotrep")
                            nc.tensor.matmul(
                                rot_ps[:], P_t[:], q_sb[:], start=True, stop=True
                            )
                            rep_ps = psB.tile([128, 512], F32, tag="rotrep")
                            nc.tensor.matmul(
                                rep_ps[:],
                                ind16_t[:, jt * 128 : (jt + 1) * 128],
                                rs16[:],
                                start=True,
                                stop=True,
                            )
                            t1 = sbB.tile([128, 512], F16, tag="t1")
                            nc.vector.tensor_tensor(
                                t1[:], q_sb[:], cosF_t[:, cs], MUL
                            )
                            t2 = sbB.tile([128, 512], F16, tag="t2")
                            nc.vector.tensor_tensor(
                                t2[:], rot_ps[:], sinF_t[:, cs], MUL
                            )
                            s = sbB.tile([128, 512], F16, tag="sB")
                            nc.vector.tensor_tensor(s[:], t1[:], t2[:], ADD)
                            nc.vector.tensor_tensor(
                                qh_all[:, jt * 512 : (jt + 1) * 512],
                                s[:],
                                rep_ps[:],
                                MUL,
                            )
                        qh_tiles.append((cs, qh_all))

                    # ====== AR-dependent: blockdiag unpack + G = vk@WoT =====
                    # the unpack copies ride the DVE queue at B-end, when the
                    # AR (and kvp load) have long completed
                  if "C" in phases and "B" in phases:
                    nc.scalar.activation(
                        vkblk[0:64, :].rearrange("p (t f) -> p t f", t=NPAIR)[
                            :, :, 0:64
                        ],
                        kvp[0:64, :].rearrange("p (t f) -> p t f", t=NPAIR),
                        Copy,
                    )
                    nc.scalar.activation(
                        vkblk[64:128, :].rearrange("p (t f) -> p t f", t=NPAIR)[
                            :, :, 64:128
                        ],
                        kvp[64:128, :].rearrange("p (t f) -> p t f", t=NPAIR),
                        Copy,
                    )

                    G_sb = kvblk_pool.tile(
                        [128, NPAIR * DIM], F8 if fp8o else F16, tag="G_sb"
                    )
                    for p in range(NPAIR):
                        for half in range(2):
                            g_ps = psO.tile([128, 512], F32, tag="o_ps")
                            nc.tensor.matmul(
                                g_ps[:],
                                vkblk[:, p * 128 : (p + 1) * 128],
                                wo_all[
                                    :,
                                    p * DIM
                                    + half * 512 : p * DIM
                                    + half * 512
                                    + 512,
                                ],
                                start=True,
                                stop=True,
                            )
                            gdst = G_sb[
                                :,
                                p * DIM + half * 512 : p * DIM + half * 512 + 512,
                            ]
                            nc.scalar.activation(
                                gdst, g_ps[:], Copy, scale=XS_G if fp8o else 1.0
                            )

                    # ============= Phase C: out = G^T q, store ==============
                    out_tgt = scratch[_rep] if _rep < reps - 1 else out_d
                    for cs, qh_all in qh_tiles:
                        if big_store:
                            o_all = sbO.tile([128, NJ * 512], F16, tag="o_all")
                        for et in range(NJ):
                            elo = et * 128
                            o_ps = psO.tile([128, 512], F32, tag="o_ps")
                            if fp8o:
                                for p in range(0, NPAIR, 2):
                                    nc.tensor.matmul(
                                        o_ps[:],
                                        G_sb[
                                            :, p * DIM : (p + 2) * DIM
                                        ].rearrange("p (t f) -> p t f", t=2)[
                                            :, :, elo : elo + 128
                                        ],
                                        qh_all[
                                            :, p * 512 : (p + 2) * 512
                                        ].rearrange("p (t c) -> p t c", t=2),
                                        start=(p == 0),
                                        stop=(p == NPAIR - 2),
                                        perf_mode=DoubleRow,
                                    )
                            else:
                                for p in range(NPAIR):
                                    nc.tensor.matmul(
                                        o_ps[:],
                                        G_sb[
                                            :, p * DIM + elo : p * DIM + elo + 128
                                        ],
                                        qh_all[:, p * 512 : (p + 1) * 512],
                                        start=(p == 0),
                                        stop=(p == NPAIR - 1),
                                    )
                            if big_store:
                                o_sb = o_all[:, et * 512 : (et + 1) * 512]
                            else:
                                o_t = sbO.tile([128, 512], F16, tag="o_sb")
                                o_sb = o_t[:]
                            nc.scalar.activation(
                                o_sb,
                                o_ps[:],
                                Copy,
                                scale=1.0 / (XS_G * XS_Q) if fp8o else 1.0,
                            )
                            if not big_store:
                                # store each 128-row chunk as it's ready
                                nc.scalar.dma_start(
                                    out=out_tgt[elo : elo + 128, cs],
                                    in_=o_sb,
                                )
                        if big_store:
                            nc.scalar.dma_start(
                                out=blkview(out_tgt, cs),
                                in_=o_all[:].rearrange(
                                    "p (t c) -> p t c", t=NJ
                                ),
                            )

            # keep-alive reads for the scratch outputs (timing builds only)
            for r, scr in enumerate(scratch):
                ka = consts.tile([16, 16], F16, tag=f"ka{r}")
                nc.sync.dma_start(out=ka[:], in_=scr[0:16, 0:16])
                nc.sync.dma_start(
                    out=out_d[0:16, r * 16 : (r + 1) * 16], in_=ka[:]
                )
            # phase-subset timing builds: touch params the skipped phases
            # would read, so every variant keeps the same param list
            if phases != "ABC":
                for i, ap in enumerate((WqT, WoT, cosF, sinF)):
                    kb = consts.tile([16, 16], ap.tensor.dtype, tag=f"kb{i}")
                    nc.sync.dma_start(out=kb[:], in_=ap[0:16, 0:16])
                kc = consts.tile([16, 16], F16, tag="kc")
                nc.vector.memset(kc[:], 0.0)
                nc.sync.dma_start(out=out_d[16:32, 0:16], in_=kc[:])

    nc.compile()
    return nc


_NC_CACHE = None


def _get_nc():
    global _NC_CACHE
    if _NC_CACHE is None:
        _NC_CACHE = build_nc()
    return _NC_CACHE


def make_in_maps(x, mask, Wq, Wk, Wv, Wo, norm_const):
    x = np.asarray(x, np.float32)
    mask = np.asarray(mask)
    Wq = np.asarray(Wq, np.float32)
    Wk = np.asarray(Wk, np.float32)
    Wv = np.asarray(Wv, np.float32)
    Wo = np.asarray(Wo, np.float32)
    norm_const = np.asarray(norm_const, np.float32).reshape(H)

    sig = 1.0 / (1.0 + np.exp(-norm_const.astype(np.float64)))
    svec = np.float64(C) ** (-sig)  # [H]
    s_cols = np.repeat(svec, HD)  # [DIM]

    f16 = np.float16
    f8 = mybir.dt.np(F8)
    WvTf = np.ascontiguousarray((Wv * s_cols[:, None].astype(np.float32)).T)
    WkT = np.ascontiguousarray(Wk.T).astype(f16)
    WvT = WvTf.astype(f16)
    WqT = np.ascontiguousarray(Wq.T).astype(f16)
    WoT = np.ascontiguousarray(Wo.T).astype(f16)
    Wk8 = (Wk.T * XS_K).astype(f8)
    Wq8 = (Wq.T * XS_K).astype(f8)
    Wv8 = (WvTf * XS_V).astype(f8)

    inv_freq = 1.0 / (
        ROPE_THETA ** (np.arange(0, HD, 2, dtype=np.float64) / HD)
    )  # [32]
    freq_of_j = np.repeat(inv_freq, 2)  # [64] interleaved

    ind16T = np.zeros((DIM, 16), f16)
    for jt in range(NJ):
        for kk in range(128):
            ind16T[jt * 128 + kk, 2 * jt + (kk >= 64)] = 1.0

    ind16 = np.zeros((16, DIM), f16)
    for jt in range(NJ):
        for m in range(128):
            ind16[2 * jt + (m >= 64), jt * 128 + m] = 1.0

    Pmat = np.zeros((128, 128), f16)
    for i in range(64):
        Pmat[2 * i + 1, 2 * i] = -1.0  # out[2i] = -q[2i+1]
        Pmat[2 * i, 2 * i + 1] = 1.0  # out[2i+1] = q[2i]

    in_maps = []
    for core in range(N_CORES):
        b = core // (N_CORES // B)
        cc = core % (N_CORES // B)
        c0 = cc * R
        pos = (c0 + np.arange(R)).astype(np.float64)

        xTc = np.ascontiguousarray(x[b, c0 : c0 + R, :].T).astype(f16)

        angC = pos[:, None] * freq_of_j[None, :]  # [R, 64]
        cosCc = np.cos(angC).astype(f16)
        sinCc = np.sin(angC).astype(np.float32)
        # sign fold for the swap formulation: even j -> -sin, odd j -> +sin
        sinCc[:, 0::2] *= -1.0
        sinCc = sinCc.astype(f16)

        angF = freq_of_j[:, None] * pos[None, :]  # [64, R]
        angF2 = np.concatenate([angF, angF], axis=0)  # [128, R]
        cosFc = np.cos(angF2).astype(f16)
        sinFc = np.sin(angF2).astype(f16)

        mrow = mask[b, c0 : c0 + R].astype(np.float32)  # [R]
        maskCc = np.ascontiguousarray(mrow.reshape(NC_T, 128).T)  # [128, NC_T]

        in_maps.append(
            {
                "xT": xTc,
                "xT8": xTc.astype(f8),
                "WkT": WkT,
                "WvT": WvT,
                "WqT": WqT,
                "WoT": WoT,
                "Wk8": Wk8,
                "Wq8": Wq8,
                "Wv8": Wv8,
                "maskCv": maskCc / XS_V,
                "cosC": cosCc,
                "sinC": sinCc,
                "cosF": cosFc,
                "sinF": sinFc,
                "maskC": maskCc,
                "ind16T": ind16T,
                "ind16": ind16,
                "Pmat": Pmat,
            }
        )
    return in_maps


def assemble_output(results, mask):
    out = np.empty((B, C, DIM), np.float32)
    for core in range(N_CORES):
        b = core // (N_CORES // B)
        cc = core % (N_CORES // B)
        c0 = cc * R
        out[b, c0 : c0 + R, :] = results[core]["out"].T.astype(np.float32)
    # q-side mask: masked rows produce zero output
    out *= np.asarray(mask)[:, :, None].astype(np.float32)
    return out


def kernel(x, mask, Wq, Wk, Wv, Wo, norm_const):
    nc = _get_nc()
    in_maps = make_in_maps(x, mask, Wq, Wk, Wv, Wo, norm_const)
    res = run_bass_kernel_spmd(nc, in_maps, list(range(N_CORES)))
    return assemble_output(res.results, mask)
